# revision 20
# baseline (speedup 1.0000x reference)
"""Trainium2 Bass kernel: 16-head causal attention with sink logit.

Contract: kernel(**inputs) takes the FULL inputs of the reference
(x [2,2048,1024], W_Q/W_K/W_V/W_out [1024,1024], sink [16]) and returns
the FULL output [2,2048,1024], running on 8 NeuronCores.

Sharding: core c = b*4 + g handles batch b and heads [4g, 4g+4).
Each core computes yT_partial [1024, 2048] = W_out_slice^T @ attn^T;
host sums the 4 partials per batch and transposes.

v3: baseline QC=512 software-pipelined structure with everything in
bf16 (host pre-converts x and the weight slices), batched [128,1024]
exp from PSUM, single broadcast-MM norm per (p,qc) deferred off the PE
critical path, and one-bank-per-accumulation-group PSUM discipline.
"""

import sys
import numpy as np

if "/opt/trn_rl_repo" not in sys.path:
    sys.path.insert(0, "/opt/trn_rl_repo")

B, T, C = 2, 2048, 1024
H, D = 16, 64
G = 4                # heads per core
DH = G * D           # 256 head-dims per core
NCORES = 8
QC = 512             # q chunk
NQ = T // QC         # 4
NKT = T // 128       # 16 k-tiles
NCC = C // 128       # 8 contraction chunks over C
SCALE = 1.0 / float(np.sqrt(D))

# vp per-kt slot layout (386 cols per kt), bf16:
#   pair p head A: [V(64) | one]             off p*193+0,  width 65,  denom row 64
#   pair p head B: [one | zeros(63) | V(64)] off p*193+65, width 128, denom row 0
VP_W = 386
VP_OFF = [0, 65, 193, 258]


def build_program(reps=1):
    """Build the per-core Bass program. reps>1 repeats the compute body
    (same inputs -> same outputs) for differential wall-clock timing."""
    from contextlib import ExitStack

    import concourse.bass as bass
    import concourse.tile as tile
    from concourse import bacc, mybir

    f32 = mybir.dt.float32
    f32r = mybir.dt.float32r
    bf16 = mybir.dt.bfloat16
    AF = mybir.ActivationFunctionType
    Alu = mybir.AluOpType

    nc = bacc.Bacc("TRN2", target_bir_lowering=False, debug=False)

    xt_d = nc.dram_tensor("xt", [C, T], bf16, kind="ExternalInput").ap()
    wq_d = nc.dram_tensor("wq", [C, DH], bf16, kind="ExternalInput").ap()
    wk_d = nc.dram_tensor("wk", [C, DH], bf16, kind="ExternalInput").ap()
    wv_d = nc.dram_tensor("wv", [C, DH], bf16, kind="ExternalInput").ap()
    wo_d = nc.dram_tensor("wo", [DH, C], bf16, kind="ExternalInput").ap()
    sk_d = nc.dram_tensor("sk", [1, G], f32, kind="ExternalInput").ap()
    cm_d = nc.dram_tensor("cm", [128, 4096], bf16, kind="ExternalInput").ap()
    vpc_d = nc.dram_tensor("vpc", [128, NKT * 65], bf16, kind="ExternalInput").ap()
    ind_d = nc.dram_tensor("ind", [128, 128], f32r, kind="ExternalInput").ap()
    zr_d = nc.dram_tensor("zr", [128, QC], f32r, kind="ExternalInput").ap()
    yt_d = nc.dram_tensor("yt", [C, T], f32, kind="ExternalOutput").ap()

    xt_v = xt_d.rearrange("(n p) m -> p n m", p=128)   # [128, 8, 2048]
    wq_v = wq_d.rearrange("(n p) m -> p n m", p=128)   # [128, 8, 256]
    wk_v = wk_d.rearrange("(n p) m -> p n m", p=128)
    wv_v = wv_d.rearrange("(n p) m -> p n m", p=128)
    wo_v = wo_d.rearrange("(n p) m -> p n m", p=128)   # [128, 2, 1024]
    yt_v = yt_d.rearrange("(n p) m -> p n m", p=128)   # [128, 8, 2048]

    with tile.TileContext(nc) as tc, ExitStack() as ctx:
        P = lambda name, bufs: ctx.enter_context(tc.tile_pool(name=name, bufs=bufs))
        const_p = P("const", 1)
        big_p = P("big", 1)
        p_p = P("p", 4)
        y_p = P("y", 2)
        row_p = P("row", 2)
        ps_p = ctx.enter_context(tc.tile_pool(name="ps", bufs=2, space="PSUM"))
        o_p = ctx.enter_context(tc.tile_pool(name="o", bufs=2, space="PSUM"))

        # ---- persistent SBUF tensors ----
        xt_sb = big_p.tile([128, NCC * T], bf16, tag="xt")           # 32KB/part
        wq_sb = big_p.tile([128, NCC * DH], bf16, tag="wq")
        wk_sb = big_p.tile([128, NCC * DH], bf16, tag="wk")
        wv_sb = big_p.tile([128, NCC * DH], bf16, tag="wv")
        wo_sb = big_p.tile([128, 2 * C], bf16, tag="wo")
        qt_sb = big_p.tile([128, 2 * T], bf16, tag="qt")
        kt_sb = big_p.tile([128, 2 * T], bf16, tag="kt")
        vp_sb = big_p.tile([128, NKT * VP_W], bf16, tag="vp")
        at_sb = big_p.tile([128, 2 * T], bf16, tag="at")             # attn^T normalized
        rc_sb = big_p.tile([128, QC], f32r, tag="rc")
        cm_sb = const_p.tile([128, 4096], bf16, tag="cm")
        ind_sb = const_p.tile([128, 128], f32r, tag="ind")
        skr_sb = const_p.tile([128, G], f32, tag="skr")
        esk_sb = const_p.tile([128, G], f32, tag="esk")

        # ---- one-time loads + constants (outside reps) ----
        for i in range(NCC):
            nc.sync.dma_start(xt_sb[:, i * T:(i + 1) * T], xt_v[:, i, :])
        nc.sync.dma_start(
            wq_sb[:].rearrange("p (n m) -> p n m", m=DH), wq_v[:, :, :])
        nc.sync.dma_start(
            wk_sb[:].rearrange("p (n m) -> p n m", m=DH), wk_v[:, :, :])
        nc.sync.dma_start(
            wv_sb[:].rearrange("p (n m) -> p n m", m=DH), wv_v[:, :, :])
        nc.sync.dma_start(
            wo_sb[:].rearrange("p (n m) -> p n m", m=C), wo_v[:, :, :])
        nc.sync.dma_start(cm_sb[:, :], cm_d[:, :])
        nc.sync.dma_start(skr_sb[0:1, :], sk_d[:, :])
        nc.sync.dma_start(skr_sb[64:65, :], sk_d[:, :])
        nc.scalar.activation(esk_sb[0:1, :], skr_sb[0:1, :], AF.Exp)
        nc.scalar.activation(esk_sb[64:65, :], skr_sb[64:65, :], AF.Exp)
        vp_view = vp_sb[:].rearrange("p (k w) -> p k w", w=VP_W)
        vpc_view = vpc_d.rearrange("p (k w) -> p k w", w=65)
        nc.sync.dma_start(vp_view[:, :, 64:129], vpc_view[:, :, :])
        nc.sync.dma_start(vp_view[:, :, 257:322], vpc_view[:, :, :])
        nc.sync.dma_start(ind_sb[:, :], ind_d[:, :])
        nc.sync.dma_start(rc_sb[:, :], zr_d[:, :])

        for _ in range(reps):
            # ---- phase 1: Q^T and K^T projections  [d(128/pair), t] ----
            for w_sb, t_sb in ((wq_sb, qt_sb), (wk_sb, kt_sb)):
                for mt in range(2):           # head pair -> 128 d rows
                    for qp in range(NQ // 2):
                        ps = ps_p.tile([128, 2 * QC], f32, tag="ps")
                        for half in range(2):
                            qc = qp * 2 + half
                            for ci in range(NCC):
                                nc.tensor.matmul(
                                    ps[:, half * QC:(half + 1) * QC],
                                    w_sb[:, ci * DH + mt * 128: ci * DH + (mt + 1) * 128],
                                    xt_sb[:, ci * T + qc * QC: ci * T + qc * QC + QC],
                                    start=(ci == 0), stop=(ci == NCC - 1))
                        nc.vector.tensor_copy(
                            t_sb[:, mt * T + qp * 2 * QC: mt * T + (qp + 1) * 2 * QC],
                            ps[:, :])

            # ---- phase 1b: V natural [t, d] into padded vp layout ----
            for tq in range(NKT // 4):
                ps = ps_p.tile([128, 2 * QC], f32, tag="ps")
                for sub in range(4):
                    tt = tq * 4 + sub
                    for ci in range(NCC):
                        # 2 tt per bank: single accumulation group per bank
                        nc.tensor.matmul(
                            ps[:, sub * DH:(sub + 1) * DH],
                            xt_sb[:, ci * T + tt * 128: ci * T + (tt + 1) * 128],
                            wv_sb[:, ci * DH: (ci + 1) * DH],
                            start=(sub % 2 == 0 and ci == 0),
                            stop=(sub % 2 == 1 and ci == NCC - 1))
                for sub in range(4):
                    tt = tq * 4 + sub
                    base = tt * VP_W
                    s0 = sub * DH
                    nc.vector.tensor_copy(vp_sb[:, base + 0: base + 64], ps[:, s0:s0 + 64])
                    nc.vector.tensor_copy(vp_sb[:, base + 129: base + 257], ps[:, s0 + 64:s0 + 192])
                    nc.vector.tensor_copy(vp_sb[:, base + 322: base + 386], ps[:, s0 + 192:s0 + 256])

            # ---- phase 2+3: attention per q-chunk + output projection,
            # software-pipelined exactly like the baseline: scores(kt)
            # are emitted before PV(kt-1); normalize / output-projection
            # blocks are deferred into later kt loops.
            deferred = []

            def emit_scores(p, qc, kt):
                sAB = ps_p.tile([128, 2 * QC], f32, tag="ps")
                nc.tensor.matmul(
                    sAB[:, 0:QC],
                    kt_sb[0:64, p * T + kt * 128: p * T + (kt + 1) * 128],
                    qt_sb[0:64, p * T + qc * QC: p * T + qc * QC + QC],
                    start=True, stop=True)
                nc.tensor.matmul(
                    sAB[:, QC:2 * QC],
                    kt_sb[64:128, p * T + kt * 128: p * T + (kt + 1) * 128],
                    qt_sb[64:128, p * T + qc * QC: p * T + qc * QC + QC],
                    start=True, stop=True)
                diag = kt - 4 * qc
                pAB = p_p.tile([128, 2 * QC], bf16, tag="p")
                nc.scalar.activation(pAB[:, :], sAB[:, :], AF.Exp, scale=SCALE)
                if diag >= 0:
                    with nc.allow_low_precision(reason="0/1 mask mult"):
                        nc.vector.tensor_mul(pAB[:, :], pAB[:, :],
                                             cm_sb[:, diag * 1024:(diag + 1) * 1024])
                return pAB

            def emit_pv(p, qc, kt, nkt, oAB, pAB):
                base = kt * VP_W
                # A accumulates in bank 0 (cols 0:512), B in bank 1
                nc.tensor.matmul(
                    oAB[0:65, 0:QC],
                    vp_sb[:, base + VP_OFF[2 * p]: base + VP_OFF[2 * p] + 65],
                    pAB[:, 0:QC],
                    start=(kt == 0), stop=(kt == nkt - 1))
                nc.tensor.matmul(
                    oAB[:, QC:2 * QC],
                    vp_sb[:, base + VP_OFF[2 * p + 1]: base + VP_OFF[2 * p + 1] + 128],
                    pAB[:, QC:2 * QC],
                    start=(kt == 0), stop=(kt == nkt - 1))

            def make_denoms(p, qc, oAB):
                def emit():
                    hA, hB = 2 * p, 2 * p + 1
                    dn = row_p.tile([128, QC], f32, tag="row")
                    nc.vector.tensor_scalar(
                        out=dn[64:65, :], in0=oAB[64:65, 0:QC],
                        scalar1=esk_sb[64:65, hA:hA + 1], scalar2=None, op0=Alu.add)
                    nc.vector.tensor_scalar(
                        out=dn[0:1, :], in0=oAB[0:1, QC:2 * QC],
                        scalar1=esk_sb[0:1, hB:hB + 1], scalar2=None, op0=Alu.add)
                    with nc.allow_low_precision(reason="f32r recip for PE broadcast"):
                        nc.vector.reciprocal(rc_sb[64:65, :], dn[64:65, :])
                        nc.vector.reciprocal(rc_sb[0:1, :], dn[0:1, :])
                return emit

            def make_normalize(p, qc, oAB):
                def emit():
                    bc = ps_p.tile([128, 2 * QC], f32, tag="ps")
                    nc.tensor.matmul(
                        bc[:, 0:QC], ind_sb[:, :], rc_sb[:, :],
                        start=True, stop=True)
                    bcs = row_p.tile([128, QC], f32, tag="bcs")
                    nc.vector.tensor_copy(bcs[:, :], bc[:, 0:QC])
                    cs = slice(p * T + qc * QC, p * T + qc * QC + QC)
                    with nc.allow_low_precision(reason="bf16 attn out"):
                        nc.vector.tensor_mul(at_sb[0:64, cs],
                                             oAB[0:64, 0:QC], bcs[0:64, :])
                        nc.vector.tensor_mul(at_sb[64:128, cs],
                                             oAB[64:128, QC:2 * QC], bcs[64:128, :])
                return emit

            def make_wout(qc, cop):
                def emit():
                    ps = ps_p.tile([128, 2 * QC], f32, tag="ps")
                    for half in range(2):
                        co = cop * 2 + half
                        for j in range(2):
                            nc.tensor.matmul(
                                ps[:, half * QC:(half + 1) * QC],
                                wo_sb[:, j * C + co * 128: j * C + (co + 1) * 128],
                                at_sb[:, j * T + qc * QC: j * T + qc * QC + QC],
                                start=(j == 0), stop=(j == 1))
                    yt = y_p.tile([128, 2 * QC], f32, tag="y")
                    nc.vector.tensor_copy(yt[:, :], ps[:, :])
                    nc.sync.dma_start(
                        yt_v[:, cop * 2: cop * 2 + 2, qc * QC: qc * QC + QC],
                        yt[:, :].rearrange("p (n m) -> p n m", m=QC))
                return emit

            for qc in range(NQ):
                nkt = 4 * qc + 4
                for p in range(2):
                    oAB = o_p.tile([128, 2 * QC], f32, tag="o")
                    prev = emit_scores(p, qc, 0)
                    for kt in range(1, nkt):
                        cur = emit_scores(p, qc, kt)
                        if kt >= 2 and deferred:
                            deferred.pop(0)()
                        emit_pv(p, qc, kt - 1, nkt, oAB, prev)
                        prev = cur
                    emit_pv(p, qc, nkt - 1, nkt, oAB, prev)
                    # denominators early (DVE), broadcast+normalize deferred
                    deferred.append(make_denoms(p, qc, oAB))
                    deferred.append(make_normalize(p, qc, oAB))
                for cop in range(NCC // 2):
                    deferred.append(make_wout(qc, cop))
            for fn in deferred:
                fn()
            deferred.clear()

    nc.compile()
    return nc


def make_causal_masks():
    """cm [128, 4096] bf16: 4 diagonal masks m=0..3, each [q>=k+128m]
    duplicated for the A|B halves of the [128,1024] pAB tile."""
    import ml_dtypes
    kl = np.arange(128)[:, None]
    ql = np.arange(QC)[None, :]
    cm = np.zeros((128, 4096), dtype=np.float32)
    for m in range(4):
        pm = (ql >= kl + 128 * m).astype(np.float32)
        cm[:, m * 1024: m * 1024 + QC] = pm
        cm[:, m * 1024 + QC: (m + 1) * 1024] = pm
    return cm.astype(ml_dtypes.bfloat16)


def shard_inputs(x, W_Q, W_K, W_V, W_out, sink):
    import ml_dtypes
    bf = ml_dtypes.bfloat16
    cm = make_causal_masks()
    vpc = np.zeros((128, 65), dtype=np.float32)
    vpc[:, 0:2] = 1.0
    vpc = np.tile(vpc, (1, NKT)).astype(bf)
    ind = np.zeros((128, 128), dtype=np.float32)
    ind[64, 0:64] = 1.0   # head A recip (row 64) -> rows 0-63
    ind[0, 64:128] = 1.0  # head B recip (row 0) -> rows 64-127
    in_maps = []
    for c in range(NCORES):
        b, g = divmod(c, G)
        cols = slice(g * DH, (g + 1) * DH)
        in_maps.append({
            "xt": np.ascontiguousarray(x[b].T).astype(bf),
            "wq": np.ascontiguousarray(W_Q[:, cols]).astype(bf),
            "wk": np.ascontiguousarray(W_K[:, cols]).astype(bf),
            "wv": np.ascontiguousarray(W_V[:, cols]).astype(bf),
            "wo": np.ascontiguousarray(W_out[cols, :]).astype(bf),
            "sk": np.ascontiguousarray(sink[g * G:(g + 1) * G][None, :]),
            "cm": cm,
            "vpc": vpc,
            "ind": ind,
            "zr": np.zeros((128, QC), dtype=np.float32),
        })
    return in_maps


def gather_outputs(results):
    out = np.zeros((B, T, C), dtype=np.float32)
    for b in range(B):
        acc = np.zeros((C, T), dtype=np.float32)
        for g in range(G):
            acc += results[b * G + g]["yt"]
        out[b] = acc.T
    return out


_CACHE = {}


def _get_program():
    if "nc" not in _CACHE:
        _CACHE["nc"] = build_program(reps=1)
    return _CACHE["nc"]


def kernel(x, W_Q, W_K, W_V, W_out, sink):
    from concourse.bass_utils import run_bass_kernel_spmd

    x = np.asarray(x, dtype=np.float32)
    W_Q = np.asarray(W_Q, dtype=np.float32)
    W_K = np.asarray(W_K, dtype=np.float32)
    W_V = np.asarray(W_V, dtype=np.float32)
    W_out = np.asarray(W_out, dtype=np.float32)
    sink = np.asarray(sink, dtype=np.float32)

    nc = _get_program()
    in_maps = shard_inputs(x, W_Q, W_K, W_V, W_out, sink)
    res = run_bass_kernel_spmd(nc, in_maps, core_ids=list(range(NCORES)))
    return gather_outputs(res.results)
